# revision 8
# baseline (speedup 1.0000x reference)
"""HeteroGATConv Trainium2 kernel (8 NeuronCores, SPMD, no collectives).

Strategy:
  - 4 relations (A->T, T->A, A->T, T->A); each relation's dst nodes are
    partitioned into 16 shards (2 per core). All edges are routed (on host)
    to the core owning their dst, so segment-softmax and segment-sum are
    core-local -- no collectives.
  - Host bin-packs dst nodes into tiles of 128 slots such that every tile
    holds at most B*128 edges; each tile's edge list is padded to exactly
    B*128 with sentinel edges. This makes the per-core instruction stream
    identical (uniform SPMD program), with data-driven behavior only.
  - Device per job (relation x shard): transform the src-halo rows into a
    fused bf16 DRAM table  row = [xs(256) | al_s(4) | pad->384]  (768B,
    dma_gather-aligned); al_d into a second bf16 table (256B rows).  Per
    edge superblock, dma_gather the rows, compute p = exp(leakyrelu(
    al_s+al_d)), build one-hot(dst-slot) via is_equal against an iota row,
    and matmul-accumulate [p*xs | p] (bf16) into an fp32 PSUM tile indexed
    by dst slot.  PSUM holds [Num | Den] per dst; flush = Num/(Den+eps)
    summed over heads into an fp32 SBUF accumulator.  Final: *0.125 (mean
    over 4 heads and 2 relations) + bias, DMA out.
"""

import sys

sys.path.insert(0, "/opt/trn_rl_repo")

import ml_dtypes
import numpy as np
from contextlib import ExitStack

import concourse.bacc as bacc
import concourse.bass as bass
import concourse.mybir as mybir
import concourse.tile as tile
from concourse.bass_utils import run_bass_kernel_spmd
from concourse.library_config import mlp

F32 = mybir.dt.float32
BF16 = mybir.dt.bfloat16
I16 = mybir.dt.int16
ALU = mybir.AluOpType
ACTF = mybir.ActivationFunctionType
NPBF = ml_dtypes.bfloat16

P = 128          # partitions / dst slots per tile / edges per block
D_IN = 128
H, C = 4, 64
HC = H * C       # 256
ROW = HC + H     # 260 = [Num | Den] free width
TROW = 384       # fused table row (bf16): [xs 256 | al_s 4 | pad] = 768 B
AROW = 128       # al_d table row (bf16): [al_d 4 | pad] = 256 B
SB_BLOCKS = 8    # edge blocks per dma_gather superblock (1024 idxs; 2048 overflows the SWDGE descriptor ring)
GA = 8           # phase-A chunk group (batched DMA)
NEG_SLOPE = 0.2
EPS = 1e-16

# relation -> (src type, dst type)
REL_TYPES = [("a", "t"), ("t", "a"), ("a", "t"), ("t", "a")]


# ----------------------------------------------------------------------------
# Host-side preparation
# ----------------------------------------------------------------------------

def _pack_dsts(deg_comb, n_shards):
    """Assign dst ids to (shard, tile, slot) balancing edges.

    Returns packed_id[d] = shard*(T*128) + tile*128 + slot,
    dst_of_slot[shard, T*128] (-1 for empty), T.
    """
    nd = deg_comb.shape[0]
    assert nd % n_shards == 0
    nd_shard = nd // n_shards
    T = -(-nd_shard // P)
    order = np.argsort(-deg_comb, kind="stable")
    sh_of = np.empty(nd, np.int32)
    for i in range(n_shards):
        fwd = order[i::2 * n_shards]
        bwd = order[2 * n_shards - 1 - i::2 * n_shards]
        mine = np.concatenate([fwd, bwd])
        sh_of[mine] = i
    packed = np.full(nd, -1, np.int64)
    dst_of_slot = np.full((n_shards, T * P), -1, np.int64)
    for s in range(n_shards):
        mine = order[sh_of[order] == s]  # desc degree within shard
        tile_of = np.empty(len(mine), np.int32)
        slot_in = np.empty(len(mine), np.int32)
        fill = np.zeros(T, np.int32)
        ti, direction = 0, 1
        for k, d in enumerate(mine):
            while fill[ti] >= P:
                ti += direction
                if ti == T or ti < 0:
                    direction = -direction
                    ti += direction
            tile_of[k] = ti
            slot_in[k] = fill[ti]
            fill[ti] += 1
            ti += direction
            if ti == T or ti < 0:
                direction = -direction
                ti += direction
        g = s * T * P + tile_of * P + slot_in
        packed[mine] = g
        dst_of_slot[s, tile_of * P + slot_in] = mine
    return packed, dst_of_slot, T


def _wrap_idx(arr_i16):
    """[E] -> [128, E/16] int16 wrapped in 16 partitions, replicated x8."""
    e = arr_i16.shape[0]
    assert e % 16 == 0
    w = arr_i16.reshape(e // 16, 16).T  # [16, E/16]
    return np.tile(w, (8, 1)).astype(np.int16)


def prep(inputs, n_cores=8):
    """Build per-core input maps + metadata for the kernel builder."""
    x = {"a": np.asarray(inputs["x_agent"], np.float32),
         "t": np.asarray(inputs["x_track"], np.float32)}
    W = np.asarray(inputs["W"], np.float32)          # [4, 128, 256]
    att_src = np.asarray(inputs["att_src"], np.float32)
    att_dst = np.asarray(inputs["att_dst"], np.float32)
    bias = np.asarray(inputs["bias"], np.float32)    # [4, 64]
    edges = [(np.asarray(inputs[f"e{r}_src"]), np.asarray(inputs[f"e{r}_dst"]))
             for r in range(4)]

    n_shards = 2 * n_cores
    n = {k: v.shape[0] for k, v in x.items()}

    rel_of_type = {"t": (0, 2), "a": (1, 3)}
    packed, dst_of_slot, T = {}, {}, {}
    for tau, (ra, rb) in rel_of_type.items():
        deg = (np.bincount(edges[ra][1], minlength=n[tau])
               + np.bincount(edges[rb][1], minlength=n[tau]))
        packed[tau], dst_of_slot[tau], T[tau] = _pack_dsts(deg, n_shards)

    rel_edge = []
    B = []
    for r, (st, dt) in enumerate(REL_TYPES):
        es, ed = edges[r]
        g = packed[dt][ed]
        order = np.argsort(g, kind="stable")
        es_s, g_s = es[order], g[order]
        tile_glob = g_s // P
        ntile_glob = n_shards * T[dt]
        counts = np.bincount(tile_glob, minlength=ntile_glob)
        Br = max(1, int(-(-counts.max() // P)))
        B.append(Br)
        cap = Br * P
        starts = np.concatenate([[0], np.cumsum(counts)])
        shards = []
        for s in range(n_shards):
            es_pad = np.zeros(T[dt] * cap, np.int64)
            drel = np.full(T[dt] * cap, 300.0, np.float32)
            didx = np.zeros(T[dt] * cap, np.int64)
            for t in range(T[dt]):
                tg = s * T[dt] + t
                c0, c1 = starts[tg], starts[tg + 1]
                cnt = c1 - c0
                o = t * cap
                es_pad[o:o + cnt] = es_s[c0:c1]
                drel[o:o + cnt] = (g_s[c0:c1] % P).astype(np.float32)
                didx[o:o + cnt] = g_s[c0:c1] - (s * T[dt] * P)
            uniq, inv = np.unique(
                np.concatenate([[0], es_pad]), return_inverse=True)
            es_local = inv[1:]
            shards.append(dict(es_local=es_local, drel=drel, didx=didx,
                               uniq=uniq))
        rel_edge.append(shards)

    U_pad = []
    for r in range(4):
        u = max(len(sh["uniq"]) for sh in rel_edge[r])
        U_pad.append(-(-u // (GA * P)) * (GA * P))   # multiple of 1024
        assert U_pad[r] <= 32767, f"halo too large for int16 idx: {U_pad[r]}"

    Nd_pad = {tau: T[tau] * P for tau in ("t", "a")}

    wcat, vmat = [], []
    for r in range(4):
        Wr = W[r]
        Ur = np.stack([Wr[:, h * C:(h + 1) * C] @ att_src[r, h]
                       for h in range(H)], axis=1)
        Vr = np.stack([Wr[:, h * C:(h + 1) * C] @ att_dst[r, h]
                       for h in range(H)], axis=1)
        wcat.append(np.concatenate([Wr, Ur], axis=1).astype(np.float32))
        vmat.append(Vr.astype(np.float32))

    bias_comb = {"t": np.broadcast_to(0.5 * (bias[0] + bias[2]), (P, C)).copy(),
                 "a": np.broadcast_to(0.5 * (bias[1] + bias[3]), (P, C)).copy()}
    iota = np.broadcast_to(np.arange(P).astype(NPBF), (P, P)).copy()

    meta = dict(T=T, B=B, U_pad=U_pad, Nd_pad=Nd_pad, n_cores=n_cores)

    in_maps = []
    for c in range(n_cores):
        m = {"iota": iota, "bias_t": bias_comb["t"], "bias_a": bias_comb["a"]}
        for r in range(4):
            m[f"wcat{r}"] = wcat[r]
            m[f"v{r}"] = vmat[r]
        for half in range(2):
            s = 2 * c + half
            for tau in ("t", "a"):
                rows = dst_of_slot[tau][s]
                xd = np.zeros((Nd_pad[tau], D_IN), np.float32)
                ok = rows >= 0
                xd[ok] = x[tau][rows[ok]]
                m[f"xdT_{tau}{half}"] = np.ascontiguousarray(xd.T)
        for j, (r, half) in enumerate([(r, h) for r in range(4)
                                       for h in range(2)]):
            st, dt = REL_TYPES[r]
            sh = rel_edge[r][2 * c + half]
            u = len(sh["uniq"])
            xT = np.zeros((D_IN, U_pad[r]), np.float32)
            xT[:, :u] = x[st][sh["uniq"]].T
            m[f"xT{j}"] = xT
            m[f"gidx{j}"] = _wrap_idx(sh["es_local"].astype(np.int16))
            m[f"didx{j}"] = _wrap_idx(sh["didx"].astype(np.int16))
            m[f"drel{j}"] = np.ascontiguousarray(
                sh["drel"].reshape(-1, P).T).astype(NPBF)  # [128, NBLK]
        in_maps.append(m)

    scatter = dict(dst_of_slot=dst_of_slot, T=T, n=n)
    return meta, in_maps, scatter


# ----------------------------------------------------------------------------
# Device kernel builder (uniform across cores)
# ----------------------------------------------------------------------------

def build(meta):
    T, B, U_pad, Nd_pad = meta["T"], meta["B"], meta["U_pad"], meta["Nd_pad"]

    nc = bacc.Bacc("TRN2", target_bir_lowering=False, debug=False)

    inp = {}
    inp["iota"] = nc.dram_tensor("iota", [P, P], BF16, kind="ExternalInput")
    for tau in ("t", "a"):
        inp[f"bias_{tau}"] = nc.dram_tensor(
            f"bias_{tau}", [P, C], F32, kind="ExternalInput")
        for half in range(2):
            inp[f"xdT_{tau}{half}"] = nc.dram_tensor(
                f"xdT_{tau}{half}", [D_IN, Nd_pad[tau]], F32,
                kind="ExternalInput")
    for r in range(4):
        inp[f"wcat{r}"] = nc.dram_tensor(
            f"wcat{r}", [D_IN, ROW], F32, kind="ExternalInput")
        inp[f"v{r}"] = nc.dram_tensor(
            f"v{r}", [D_IN, H], F32, kind="ExternalInput")
    jobs = [(r, h) for r in range(4) for h in range(2)]
    for j, (r, half) in enumerate(jobs):
        dt = REL_TYPES[r][1]
        nblk = T[dt] * B[r]
        inp[f"xT{j}"] = nc.dram_tensor(
            f"xT{j}", [D_IN, U_pad[r]], F32, kind="ExternalInput")
        inp[f"gidx{j}"] = nc.dram_tensor(
            f"gidx{j}", [P, nblk * 8], I16, kind="ExternalInput")
        inp[f"didx{j}"] = nc.dram_tensor(
            f"didx{j}", [P, nblk * 8], I16, kind="ExternalInput")
        inp[f"drel{j}"] = nc.dram_tensor(
            f"drel{j}", [P, nblk], BF16, kind="ExternalInput")
    out_t = nc.dram_tensor("out_t", [2 * T["t"] * P, C], F32,
                           kind="ExternalOutput")
    out_a = nc.dram_tensor("out_a", [2 * T["a"] * P, C], F32,
                           kind="ExternalOutput")

    with tile.TileContext(nc) as tc, ExitStack() as ctx:
        nc.gpsimd.load_library(mlp)

        sb_pool = ctx.enter_context(tc.tile_pool(name="sb", bufs=2))
        small = ctx.enter_context(tc.tile_pool(name="small", bufs=3))
        acc_pool = ctx.enter_context(tc.tile_pool(name="acc", bufs=1))
        cst = ctx.enter_context(tc.tile_pool(name="cst", bufs=1))
        psum = ctx.enter_context(tc.tile_pool(name="ps", bufs=6, space="PSUM"))
        pa_ps = ctx.enter_context(tc.tile_pool(name="paps", bufs=2,
                                               space="PSUM"))
        dram = ctx.enter_context(tc.tile_pool(name="dr", bufs=1,
                                              space="DRAM"))

        iota_t = cst.tile([P, P], BF16, tag="iota", name="iota_t")
        nc.sync.dma_start(out=iota_t[:], in_=inp["iota"][:])
        bias_t = {}
        for tau in ("t", "a"):
            bias_t[tau] = cst.tile([P, C], F32, tag=f"bias{tau}",
                                   name=f"bias_{tau}_t")
            nc.sync.dma_start(out=bias_t[tau][:], in_=inp[f"bias_{tau}"][:])
        wcat_t, v_t = {}, {}
        for r in range(4):
            wcat_t[r] = cst.tile([D_IN, ROW], F32, tag=f"w{r}",
                                 name=f"wcat_t{r}")
            nc.sync.dma_start(out=wcat_t[r][:], in_=inp[f"wcat{r}"][:])
            v_t[r] = cst.tile([D_IN, H], F32, tag=f"vv{r}", name=f"v_t{r}")
            nc.sync.dma_start(out=v_t[r][:], in_=inp[f"v{r}"][:])

        acc = {}
        for tau in ("t", "a"):
            acc[tau] = acc_pool.tile([P, 2 * T[tau] * C], F32,
                                     tag=f"acc{tau}", name=f"acc_{tau}")
            nc.vector.memset(acc[tau][:], 0.0)

        # ---- al_d tables per (relation, half): bf16 [Nd_pad, 128]
        ald_tab = {}
        for r in range(4):
            dt = REL_TYPES[r][1]
            for half in range(2):
                tab = dram.tile([Nd_pad[dt], AROW], BF16,
                                tag=f"ald{r}{half}", name=f"ald{r}{half}")
                xdT = inp[f"xdT_{dt}{half}"]
                nch = Nd_pad[dt] // P
                for k0 in range(0, nch, GA):
                    g = min(GA, nch - k0)
                    xbig = small.tile([D_IN, GA * P], F32, tag="xdbig",
                                      name="xdbig")
                    nc.sync.dma_start(
                        out=xbig[:, :g * P],
                        in_=xdT[:, k0 * P:(k0 + g) * P])
                    stg = small.tile([P, GA, AROW], BF16, tag="aldstg",
                                     name="aldstg")
                    nc.vector.memset(stg[:, :, H:AROW], 0.0)
                    for i in range(g):
                        ps4 = pa_ps.tile([P, H], F32, tag="pa", name="ps4")
                        nc.tensor.matmul(ps4[:],
                                         lhsT=xbig[:, i * P:(i + 1) * P],
                                         rhs=v_t[r][:], start=True, stop=True)
                        nc.scalar.activation(stg[:, i, 0:H], ps4[:],
                                             ACTF.Copy)
                    nc.sync.dma_start(
                        out=tab[k0 * P:(k0 + g) * P, :].rearrange(
                            "(i p) w -> p i w", p=P),
                        in_=stg[:, :g, :])
                ald_tab[(r, half)] = tab

        for j, (r, half) in enumerate(jobs):
            st, dt = REL_TYPES[r]
            Tt, Br = T[dt], B[r]
            nblk = Tt * Br

            # ---- phase A: fused xs|al_s table, bf16 [U_pad, 384]
            xs_tab = dram.tile([U_pad[r], TROW], BF16, tag=f"xs{st}",
                               name=f"xs_tab{j}")
            xT = inp[f"xT{j}"]
            nch = U_pad[r] // P
            for k0 in range(0, nch, GA):
                g = min(GA, nch - k0)
                xbig = small.tile([D_IN, GA * P], F32, tag="xabig",
                                  name="xabig")
                nc.sync.dma_start(out=xbig[:, :g * P],
                                  in_=xT[:, k0 * P:(k0 + g) * P])
                stg = small.tile([P, GA, TROW], BF16, tag="xastg",
                                 name="xastg")
                nc.vector.memset(stg[:, :, ROW:TROW], 0.0)
                for i in range(g):
                    psr = pa_ps.tile([P, ROW], F32, tag="pa", name="psrow")
                    nc.tensor.matmul(psr[:],
                                     lhsT=xbig[:, i * P:(i + 1) * P],
                                     rhs=wcat_t[r][:], start=True, stop=True)
                    nc.scalar.activation(stg[:, i, 0:ROW], psr[:], ACTF.Copy)
                nc.sync.dma_start(
                    out=xs_tab[k0 * P:(k0 + g) * P, :].rearrange(
                        "(i p) w -> p i w", p=P),
                    in_=stg[:, :g, :])

            # ---- load idx arrays to SBUF
            gidx_t = sb_pool.tile([P, nblk * 8], I16, tag="gidx",
                                  name="gidx_t")
            nc.sync.dma_start(out=gidx_t[:], in_=inp[f"gidx{j}"][:])
            didx_t = sb_pool.tile([P, nblk * 8], I16, tag="didx",
                                  name="didx_t")
            nc.sync.dma_start(out=didx_t[:], in_=inp[f"didx{j}"][:])
            drel_t = sb_pool.tile([P, nblk], BF16, tag="drel", name="drel_t")
            nc.sync.dma_start(out=drel_t[:], in_=inp[f"drel{j}"][:])

            # ---- phase B: edge superblocks
            ps_tile = None
            n_sb = -(-nblk // SB_BLOCKS)
            for sb in range(n_sb):
                b0 = sb * SB_BLOCKS
                nb = min(SB_BLOCKS, nblk - b0)
                ne = nb * P
                xs_g = sb_pool.tile([P, nb, TROW], BF16, tag="xsg",
                                    name="xs_g")
                nc.gpsimd.dma_gather(
                    xs_g[:], xs_tab[:], gidx_t[:, b0 * 8:(b0 + nb) * 8],
                    ne, ne, TROW)
                ald_g = sb_pool.tile([P, nb, AROW], BF16, tag="aldg",
                                     name="ald_g")
                nc.gpsimd.dma_gather(
                    ald_g[:], ald_tab[(r, half)][:],
                    didx_t[:, b0 * 8:(b0 + nb) * 8], ne, ne, AROW)

                logit = small.tile([P, nb, H], F32, tag="logit", name="logit")
                nc.vector.tensor_tensor(
                    out=logit[:], in0=xs_g[:, :, HC:HC + H],
                    in1=ald_g[:, :, 0:H], op=ALU.add)
                lrl = small.tile([P, nb, H], F32, tag="lrl", name="lrl")
                nc.vector.scalar_tensor_tensor(
                    out=lrl[:], in0=logit[:], scalar=NEG_SLOPE, in1=logit[:],
                    op0=ALU.mult, op1=ALU.max)
                p_t = small.tile([P, nb, H], BF16, tag="pt", name="p_t")
                nc.scalar.activation(p_t[:], lrl[:], ACTF.Exp)

                rhs = sb_pool.tile([P, nb, ROW], BF16, tag="rhs", name="rhs")
                for h in range(H):
                    nc.vector.tensor_tensor(
                        out=rhs[:, :, h * C:(h + 1) * C],
                        in0=xs_g[:, :, h * C:(h + 1) * C],
                        in1=p_t[:, :, h:h + 1].to_broadcast([P, nb, C]),
                        op=ALU.mult)
                nc.vector.tensor_copy(out=rhs[:, :, HC:ROW], in_=p_t[:])

                onehot = sb_pool.tile([P, nb, P], BF16, tag="oneh",
                                      name="onehot")
                nc.vector.tensor_tensor(
                    out=onehot[:],
                    in0=drel_t[:, b0:b0 + nb].unsqueeze(2).to_broadcast(
                        [P, nb, P]),
                    in1=iota_t[:].unsqueeze(1).to_broadcast([P, nb, P]),
                    op=ALU.is_equal)

                for b in range(nb):
                    g = b0 + b
                    t_idx, k = g // Br, g % Br
                    if k == 0:
                        ps_tile = psum.tile([P, ROW], F32, tag="psacc",
                                            name="ps_tile")
                    nc.tensor.matmul(ps_tile[:], lhsT=onehot[:, b, :],
                                     rhs=rhs[:, b, :], start=(k == 0),
                                     stop=(k == Br - 1))
                    if k == Br - 1:
                        den = small.tile([P, H], F32, tag="den", name="den")
                        nc.vector.tensor_scalar_add(
                            out=den[:], in0=ps_tile[:, HC:ROW], scalar1=EPS)
                        rec = small.tile([P, H], F32, tag="rec", name="rec")
                        nc.vector.reciprocal(rec[:], den[:])
                        contrib = small.tile([P, H, C], F32, tag="contrib",
                                             name="contrib")
                        nc.vector.tensor_tensor(
                            out=contrib[:],
                            in0=ps_tile[:, 0:HC].rearrange(
                                "p (h c) -> p h c", h=H),
                            in1=rec[:].unsqueeze(2).to_broadcast([P, H, C]),
                            op=ALU.mult)
                        s01 = small.tile([P, C], F32, tag="s01", name="s01")
                        nc.vector.tensor_tensor(
                            out=s01[:], in0=contrib[:, 0, :],
                            in1=contrib[:, 1, :], op=ALU.add)
                        s23 = small.tile([P, C], F32, tag="s23", name="s23")
                        nc.vector.tensor_tensor(
                            out=s23[:], in0=contrib[:, 2, :],
                            in1=contrib[:, 3, :], op=ALU.add)
                        ssum = small.tile([P, C], F32, tag="ssum",
                                          name="ssum")
                        nc.vector.tensor_tensor(
                            out=ssum[:], in0=s01[:], in1=s23[:], op=ALU.add)
                        off = (half * Tt + t_idx) * C
                        nc.vector.tensor_tensor(
                            out=acc[dt][:, off:off + C],
                            in0=acc[dt][:, off:off + C], in1=ssum[:],
                            op=ALU.add)

        # ---- finalize: out = acc * 0.125 + bias ; DMA out
        for tau, out_d in (("t", out_t), ("a", out_a)):
            nt2 = 2 * T[tau]
            nc.vector.scalar_tensor_tensor(
                out=acc[tau][:].rearrange("p (n c) -> p n c", c=C),
                in0=acc[tau][:].rearrange("p (n c) -> p n c", c=C),
                scalar=0.125,
                in1=bias_t[tau][:].unsqueeze(1).to_broadcast([P, nt2, C]),
                op0=ALU.mult, op1=ALU.add)
            nc.sync.dma_start(
                out=out_d.ap().rearrange("(n p) c -> p n c", p=P),
                in_=acc[tau][:].rearrange("p (n c) -> p n c", c=C))

    nc.compile()
    return nc


# ----------------------------------------------------------------------------
# Entry point
# ----------------------------------------------------------------------------

def kernel(**inputs):
    n_cores = 8
    meta, in_maps, scatter = prep(inputs, n_cores=n_cores)
    nc = build(meta)
    res = run_bass_kernel_spmd(nc, in_maps, list(range(n_cores)))
    results = res.results

    T, n = scatter["T"], scatter["n"]
    dst_of_slot = scatter["dst_of_slot"]
    out = {"a": np.zeros((n["a"], C), np.float32),
           "t": np.zeros((n["t"], C), np.float32)}
    for tau, name in (("a", "out_a"), ("t", "out_t")):
        for c in range(n_cores):
            rows = results[c][name]
            for half in range(2):
                s = 2 * c + half
                ids = dst_of_slot[tau][s]
                ok = ids >= 0
                seg = rows[half * T[tau] * P:(half + 1) * T[tau] * P]
                out[tau][ids[ok]] = seg[ok]
    return out["a"], out["t"]
